# revision 40
# baseline (speedup 1.0000x reference)
import sys

sys.path.insert(0, "/opt/trn_rl_repo")

import numpy as np

import concourse.bacc as bacc
import concourse.bass as bass
import concourse.mybir as mybir
import concourse.tile as tile
from concourse.bass_utils import run_bass_kernel_spmd

# Problem shapes (hardcoded per contract)
B = 4
NQ = 2048
NR = 16384
D = 64
K = 16

NCORES = 8
QPC = NQ // 2          # queries per core (each batch split across 2 cores)
NCHUNK = QPC // 128    # query chunks of 128 per core
MMN = 512              # matmul free dim (one PSUM bank of fp32)
PAIR = 2048            # refs per block (4 PSUM banks)
NPAIR = NR // PAIR     # 8
NSLOT = 4              # fp16 value slots (Act->DVE pipeline depth)
NLVL = 3               # halves-tournament levels; slot covers 2^NLVL members
NSL = PAIR >> NLVL     # 256 tournament slots per block
NW = NPAIR * 8         # 64 winner slots per query

_prog_cache = {}


def _build_program(reps: int = 1):
    if reps in _prog_cache:
        return _prog_cache[reps]

    f32 = mybir.dt.float32
    f32r = mybir.dt.float32r
    f16 = mybir.dt.float16
    u16 = mybir.dt.uint16

    nc = bacc.Bacc("TRN2", target_bir_lowering=False, debug=False, num_devices=NCORES)

    # lhsT rows 0..63 = 2*q^T, row 64 = 1.0, row 65 = q2  -> psum = 2qr - r2 - q2 = -d2
    lhs_d = nc.dram_tensor("lhs", [66, QPC], f32r, kind="ExternalInput")
    rhs_d = nc.dram_tensor("rhs", [66, NR], f32r, kind="ExternalInput")

    # top-8 tournament slots per 2048-ref block: fp16 slot max + slot index;
    # slot j of block p covers refs p*2048 + j + m*NSL (m = 0..7)
    outV_d = nc.dram_tensor("outV", [QPC, NW], f16, kind="ExternalOutput")
    outI_d = nc.dram_tensor("outI", [QPC, NW], u16, kind="ExternalOutput")

    mx = mybir.AluOpType.max

    def tt_max(out, in0, in1):
        eng = nc.vector
        return eng.add_instruction(mybir.InstTensorTensor(
            name=eng.bass.get_next_instruction_name(),
            op=mx,
            ins=[eng.lower_ap(in0), eng.lower_ap(in1)],
            outs=[eng.lower_ap(out)],
        ))

    with tile.TileContext(nc) as tc:
        with (
            tc.tile_pool(name="consts", bufs=1) as cpool,
            tc.tile_pool(name="psum", bufs=2, space="PSUM") as ppool,
            tc.tile_pool(name="work", bufs=2) as wpool,
        ):
            lhs_t = cpool.tile([66, QPC], f32r)
            rhs_t = cpool.tile([66, NR], f32r)
            # pair-0 / chunk-0 operands first so compute starts ASAP
            nc.sync.dma_start(lhs_t[:, 0:128], lhs_d.ap()[:, 0:128])
            nc.sync.dma_start(rhs_t[:, 0:PAIR], rhs_d.ap()[:, 0:PAIR])
            nc.sync.dma_start(rhs_t[:, PAIR:2 * PAIR], rhs_d.ap()[:, PAIR:2 * PAIR])
            nc.sync.dma_start(lhs_t[:, 128:QPC], lhs_d.ap()[:, 128:QPC])
            for p in range(2, NPAIR):
                c0, c1 = p * PAIR, (p + 1) * PAIR
                nc.sync.dma_start(rhs_t[:, c0:c1], rhs_d.ap()[:, c0:c1])

            # PE warmup on the tiny early lhs slice: keeps the PE busy-streak
            # alive so the first real matmuls are costed at the ramped clock
            warm = ppool.tile([128, PAIR], f32, tag="ps")
            for _ in range(4):
                nc.tensor.matmul(
                    warm[:, 0:128], lhs_t[:, 0:128], lhs_t[:, 0:128],
                    start=True, stop=True,
                )

            # trigger the activation-table load before real work
            actwarm = cpool.tile([128, 1], f32)
            nc.gpsimd.memset(actwarm[:], 0.0)
            nc.scalar.activation(
                actwarm[:], actwarm[:], mybir.ActivationFunctionType.Copy
            )

            vals = [cpool.tile([128, PAIR], f16, name=f"val{s}")
                    for s in range(NSLOT)]

            for rep in range(reps):
              for c in range(NCHUNK):
                cv = wpool.tile([128, NW], f16, tag="cv")
                ci = wpool.tile([128, NW], u16, tag="ci")
                for p in range(NPAIR):
                    ps = ppool.tile([128, PAIR], f32, tag="ps")
                    for h in range(PAIR // MMN):
                        nc.tensor.matmul(
                            ps[:, h * MMN:(h + 1) * MMN],
                            lhs_t[:, c * 128:(c + 1) * 128],
                            rhs_t[:, p * PAIR + h * MMN:p * PAIR + (h + 1) * MMN],
                            start=True,
                            stop=True,
                        )
                    # -d2 as packed fp16 values; pipeline-filling first copy
                    # goes in two contiguous pieces so it starts after mm2
                    v16 = vals[p % NSLOT]
                    if rep == 0 and c == 0 and p == 0:
                        half = PAIR // 2
                        nc.scalar.activation(
                            v16[:, 0:half], ps[:, 0:half],
                            mybir.ActivationFunctionType.Copy,
                        )
                        nc.scalar.activation(
                            v16[:, half:PAIR], ps[:, half:PAIR],
                            mybir.ActivationFunctionType.Copy,
                        )
                    else:
                        nc.scalar.activation(
                            v16[:], ps[:], mybir.ActivationFunctionType.Copy
                        )
                    # halves tournament at DVE 2x: 2048 -> 256 slot maxima
                    t1 = wpool.tile([128, PAIR // 2], f16, tag="t1")
                    tt_max(t1[:], v16[:, 0:PAIR // 2], v16[:, PAIR // 2:PAIR])
                    t2 = wpool.tile([128, PAIR // 4], f16, tag="t2")
                    tt_max(t2[:], t1[:, 0:PAIR // 4], t1[:, PAIR // 4:PAIR // 2])
                    t3 = wpool.tile([128, NSL], f16, tag="t3")
                    tt_max(t3[:], t2[:, 0:NSL], t2[:, NSL:PAIR // 4])
                    s = p * 8
                    nc.vector.max(cv[:, s:s + 8], t3[:])
                    nc.vector.max_index(ci[:, s:s + 8], cv[:, s:s + 8], t3[:])

                r0, r1 = c * 128, (c + 1) * 128
                if c == NCHUNK - 1:
                    # split the final output so the tail DMA is tiny
                    nc.sync.dma_start(outV_d.ap()[r0:r1, 0:NW // 2], cv[:, 0:NW // 2])
                    nc.sync.dma_start(outI_d.ap()[r0:r1, 0:NW // 2], ci[:, 0:NW // 2])
                    nc.sync.dma_start(outV_d.ap()[r0:r1, NW // 2:], cv[:, NW // 2:])
                    nc.sync.dma_start(outI_d.ap()[r0:r1, NW // 2:], ci[:, NW // 2:])
                else:
                    nc.sync.dma_start(outV_d.ap()[r0:r1, :], cv[:])
                    nc.sync.dma_start(outI_d.ap()[r0:r1, :], ci[:])

    nc.compile()
    _prog_cache[reps] = nc
    return nc


def kernel(ref: np.ndarray, query: np.ndarray):
    ref = np.asarray(ref, dtype=np.float32)
    query = np.asarray(query, dtype=np.float32)

    # host-side operand prep (layout + norms)
    r2 = np.sum(ref * ref, axis=-1)                      # [B, NR]
    q2 = np.sum(query * query, axis=-1)                  # [B, NQ]
    refT = np.ascontiguousarray(ref.transpose(0, 2, 1))  # [B, D, NR]
    qT = np.ascontiguousarray(query.transpose(0, 2, 1))  # [B, D, NQ]

    nc = _build_program()

    in_maps = []
    for core in range(NCORES):
        b, h = core // 2, core % 2
        lhs = np.empty((66, QPC), dtype=np.float32)
        lhs[0:D, :] = 2.0 * qT[b][:, h * QPC:(h + 1) * QPC]
        lhs[D, :] = 1.0
        lhs[D + 1, :] = q2[b, h * QPC:(h + 1) * QPC]
        rhs = np.empty((66, NR), dtype=np.float32)
        rhs[0:D, :] = refT[b]
        rhs[D, :] = -r2[b]
        rhs[D + 1, :] = -1.0
        in_maps.append({"lhs": lhs, "rhs": rhs})

    res = run_bass_kernel_spmd(nc, in_maps, core_ids=list(range(NCORES)))

    NSEL = 24                      # winner slots rescored per query (>=16 + margin)
    NMEM = 1 << NLVL               # members per slot
    base = ((np.arange(NW) >> 3) * PAIR).astype(np.int64)[None, :]
    mem = (np.arange(NMEM) * NSL).astype(np.int64)[None, None, :]
    rows = np.arange(QPC)[:, None]
    Dout = np.empty((B, NQ, K), dtype=np.float32)
    Iout = np.empty((B, NQ, K), dtype=np.int64)
    for core in range(NCORES):
        b, h = core // 2, core % 2
        v = res.results[core]["outV"].astype(np.float32)      # [QPC, NW]
        slot = base + res.results[core]["outI"].astype(np.int64)
        # top winner slots by fp16 slot-max (margin covers quantization ties)
        sel = np.argsort(-v, axis=1, kind="stable")[:, :NSEL]
        gidx = (slot[rows, sel][:, :, None] + mem).reshape(QPC, NSEL * NMEM)
        # exact rescore of all member candidates, then exact top-16
        qs = query[b, h * QPC:(h + 1) * QPC]                  # [QPC, D]
        cand = ref[b][gidx]                                   # [QPC, NSEL*NMEM, D]
        d2 = np.maximum(0.0, np.sum((cand - qs[:, None, :]) ** 2, axis=-1))
        perm = np.lexsort((gidx, d2), axis=1)[:, :K]          # (d2, idx) order
        Dout[b, h * QPC:(h + 1) * QPC] = np.sqrt(d2[rows, perm])
        Iout[b, h * QPC:(h + 1) * QPC] = gidx[rows, perm]
    return (Dout, Iout)


# revision 41
# speedup vs baseline: 1.0030x; 1.0030x over previous
import sys

sys.path.insert(0, "/opt/trn_rl_repo")

import numpy as np

import concourse.bacc as bacc
import concourse.bass as bass
import concourse.mybir as mybir
import concourse.tile as tile
from concourse.bass_utils import run_bass_kernel_spmd

# Problem shapes (hardcoded per contract)
B = 4
NQ = 2048
NR = 16384
D = 64
K = 16

NCORES = 8
QPC = NQ // 2          # queries per core (each batch split across 2 cores)
NCHUNK = QPC // 128    # query chunks of 128 per core
MMN = 512              # matmul free dim (one PSUM bank of fp32)
PAIR = 2048            # refs per block (4 PSUM banks)
NPAIR = NR // PAIR     # 8
NSLOT = 4              # fp16 value slots (Act->DVE pipeline depth)
NLVL = 3               # halves-tournament levels; slot covers 2^NLVL members
NSL = PAIR >> NLVL     # 256 tournament slots per block
NW = NPAIR * 8         # 64 winner slots per query

_prog_cache = {}


def _build_program(reps: int = 1):
    if reps in _prog_cache:
        return _prog_cache[reps]

    f32 = mybir.dt.float32
    f32r = mybir.dt.float32r
    f16 = mybir.dt.float16
    u16 = mybir.dt.uint16

    nc = bacc.Bacc("TRN2", target_bir_lowering=False, debug=False, num_devices=NCORES)

    # lhsT rows 0..63 = 2*q^T, row 64 = 1.0, row 65 = q2  -> psum = 2qr - r2 - q2 = -d2
    lhs_d = nc.dram_tensor("lhs", [66, QPC], f32r, kind="ExternalInput")
    rhs_d = nc.dram_tensor("rhs", [66, NR], f32r, kind="ExternalInput")

    # top-8 tournament slots per 2048-ref block: fp16 slot max + slot index;
    # slot j of block p covers refs p*2048 + j + m*NSL (m = 0..7)
    outV_d = nc.dram_tensor("outV", [QPC, NW], f16, kind="ExternalOutput")
    outI_d = nc.dram_tensor("outI", [QPC, NW], u16, kind="ExternalOutput")

    mx = mybir.AluOpType.max

    def tt_max(out, in0, in1):
        eng = nc.vector
        return eng.add_instruction(mybir.InstTensorTensor(
            name=eng.bass.get_next_instruction_name(),
            op=mx,
            ins=[eng.lower_ap(in0), eng.lower_ap(in1)],
            outs=[eng.lower_ap(out)],
        ))

    with tile.TileContext(nc) as tc:
        with (
            tc.tile_pool(name="consts", bufs=1) as cpool,
            tc.tile_pool(name="psum", bufs=2, space="PSUM") as ppool,
            tc.tile_pool(name="work", bufs=2) as wpool,
        ):
            lhs_t = cpool.tile([66, QPC], f32r)
            rhs_t = cpool.tile([66, NR], f32r)
            # pair-0 / chunk-0 operands first so compute starts ASAP
            nc.sync.dma_start(lhs_t[:, 0:128], lhs_d.ap()[:, 0:128])
            nc.sync.dma_start(rhs_t[:, 0:PAIR], rhs_d.ap()[:, 0:PAIR])
            nc.sync.dma_start(rhs_t[:, PAIR:2 * PAIR], rhs_d.ap()[:, PAIR:2 * PAIR])
            nc.sync.dma_start(lhs_t[:, 128:QPC], lhs_d.ap()[:, 128:QPC])
            for p in range(2, NPAIR):
                c0, c1 = p * PAIR, (p + 1) * PAIR
                nc.sync.dma_start(rhs_t[:, c0:c1], rhs_d.ap()[:, c0:c1])

            # PE warmup on the tiny early lhs slice: keeps the PE busy-streak
            # alive so the first real matmuls are costed at the ramped clock
            warm = ppool.tile([128, PAIR], f32, tag="ps")
            for _ in range(4):
                nc.tensor.matmul(
                    warm[:, 0:128], lhs_t[:, 0:128], lhs_t[:, 0:128],
                    start=True, stop=True,
                )

            # trigger the activation-table load before real work
            actwarm = cpool.tile([128, 1], f32)
            nc.gpsimd.memset(actwarm[:], 0.0)
            nc.scalar.activation(
                actwarm[:], actwarm[:], mybir.ActivationFunctionType.Copy
            )

            vals = [cpool.tile([128, PAIR], f16, name=f"val{s}")
                    for s in range(NSLOT)]

            for rep in range(reps):
              for c in range(NCHUNK):
                cv = wpool.tile([128, NW], f16, tag="cv")
                ci = wpool.tile([128, NW], u16, tag="ci")
                for p in range(NPAIR):
                    ps = ppool.tile([128, PAIR], f32, tag="ps")
                    for h in range(PAIR // MMN):
                        nc.tensor.matmul(
                            ps[:, h * MMN:(h + 1) * MMN],
                            lhs_t[:, c * 128:(c + 1) * 128],
                            rhs_t[:, p * PAIR + h * MMN:p * PAIR + (h + 1) * MMN],
                            start=True,
                            stop=True,
                        )
                    # -d2 as packed fp16 values
                    v16 = vals[p % NSLOT]
                    nc.scalar.activation(
                        v16[:], ps[:], mybir.ActivationFunctionType.Copy
                    )
                    # halves tournament at DVE 2x: 2048 -> 256 slot maxima
                    t1 = wpool.tile([128, PAIR // 2], f16, tag="t1")
                    tt_max(t1[:], v16[:, 0:PAIR // 2], v16[:, PAIR // 2:PAIR])
                    t2 = wpool.tile([128, PAIR // 4], f16, tag="t2")
                    tt_max(t2[:], t1[:, 0:PAIR // 4], t1[:, PAIR // 4:PAIR // 2])
                    t3 = wpool.tile([128, NSL], f16, tag="t3")
                    tt_max(t3[:], t2[:, 0:NSL], t2[:, NSL:PAIR // 4])
                    s = p * 8
                    nc.vector.max(cv[:, s:s + 8], t3[:])
                    nc.vector.max_index(ci[:, s:s + 8], cv[:, s:s + 8], t3[:])

                r0, r1 = c * 128, (c + 1) * 128
                if c == NCHUNK - 1:
                    # split the final output so the tail DMA is tiny
                    nc.sync.dma_start(outV_d.ap()[r0:r1, 0:NW // 2], cv[:, 0:NW // 2])
                    nc.sync.dma_start(outI_d.ap()[r0:r1, 0:NW // 2], ci[:, 0:NW // 2])
                    nc.sync.dma_start(outV_d.ap()[r0:r1, NW // 2:], cv[:, NW // 2:])
                    nc.sync.dma_start(outI_d.ap()[r0:r1, NW // 2:], ci[:, NW // 2:])
                else:
                    nc.sync.dma_start(outV_d.ap()[r0:r1, :], cv[:])
                    nc.sync.dma_start(outI_d.ap()[r0:r1, :], ci[:])

    nc.compile()
    _prog_cache[reps] = nc
    return nc


def kernel(ref: np.ndarray, query: np.ndarray):
    ref = np.asarray(ref, dtype=np.float32)
    query = np.asarray(query, dtype=np.float32)

    # host-side operand prep (layout + norms)
    r2 = np.sum(ref * ref, axis=-1)                      # [B, NR]
    q2 = np.sum(query * query, axis=-1)                  # [B, NQ]
    refT = np.ascontiguousarray(ref.transpose(0, 2, 1))  # [B, D, NR]
    qT = np.ascontiguousarray(query.transpose(0, 2, 1))  # [B, D, NQ]

    nc = _build_program()

    in_maps = []
    for core in range(NCORES):
        b, h = core // 2, core % 2
        lhs = np.empty((66, QPC), dtype=np.float32)
        lhs[0:D, :] = 2.0 * qT[b][:, h * QPC:(h + 1) * QPC]
        lhs[D, :] = 1.0
        lhs[D + 1, :] = q2[b, h * QPC:(h + 1) * QPC]
        rhs = np.empty((66, NR), dtype=np.float32)
        rhs[0:D, :] = refT[b]
        rhs[D, :] = -r2[b]
        rhs[D + 1, :] = -1.0
        in_maps.append({"lhs": lhs, "rhs": rhs})

    res = run_bass_kernel_spmd(nc, in_maps, core_ids=list(range(NCORES)))

    NSEL = 24                      # winner slots rescored per query (>=16 + margin)
    NMEM = 1 << NLVL               # members per slot
    base = ((np.arange(NW) >> 3) * PAIR).astype(np.int64)[None, :]
    mem = (np.arange(NMEM) * NSL).astype(np.int64)[None, None, :]
    rows = np.arange(QPC)[:, None]
    Dout = np.empty((B, NQ, K), dtype=np.float32)
    Iout = np.empty((B, NQ, K), dtype=np.int64)
    for core in range(NCORES):
        b, h = core // 2, core % 2
        v = res.results[core]["outV"].astype(np.float32)      # [QPC, NW]
        slot = base + res.results[core]["outI"].astype(np.int64)
        # top winner slots by fp16 slot-max (margin covers quantization ties)
        sel = np.argsort(-v, axis=1, kind="stable")[:, :NSEL]
        gidx = (slot[rows, sel][:, :, None] + mem).reshape(QPC, NSEL * NMEM)
        # exact rescore of all member candidates, then exact top-16
        qs = query[b, h * QPC:(h + 1) * QPC]                  # [QPC, D]
        cand = ref[b][gidx]                                   # [QPC, NSEL*NMEM, D]
        d2 = np.maximum(0.0, np.sum((cand - qs[:, None, :]) ** 2, axis=-1))
        perm = np.lexsort((gidx, d2), axis=1)[:, :K]          # (d2, idx) order
        Dout[b, h * QPC:(h + 1) * QPC] = np.sqrt(d2[rows, perm])
        Iout[b, h * QPC:(h + 1) * QPC] = gidx[rows, perm]
    return (Dout, Iout)
